# revision 3
# baseline (speedup 1.0000x reference)
"""Trainium2 Bass kernel for CausalSelfAttention (QK-RMSNorm + RoPE).

Sharding: 8 cores = 2 batches x 4 head-groups (4 heads each).
Each core computes QKV projection for its heads, attention, and a partial
output projection (row-parallel c_proj); host sums the 4 partials per batch
and adds b_proj.

v2 changes vs the f32r baseline:
- attention core in bf16 end to end (q/k^T, exp(S), V, O^T, c_proj weights):
  bf16 moving operands run the PE at 1 cyc/row with no small-tile penalty,
  and bf16 transposes are 1.0 cyc/row instead of 1.5.
- causal triangle applied as an additive -200 mask accumulated into the S
  PSUM tile by a pair of tiny PE matmuls (stationary = upper-triangular
  -200 matrix, moving = identity), replacing the Pool multiply on exp(S).
- QKV projection interleaves the q|k and v matmuls per contraction chunk so
  consecutive matmuls share the stationary x-block (half the Ldweights).
- output projection emits whole t-blocks (both 512-col halves) so the two
  halves share their stationary O^T blocks.
- softmax normalization multiplies PSUM operands directly (no staging copy).
- all constants packed into one bf16 DRAM parameter (wb) next to x^T: 2
  input buffers instead of 9 (each extra buffer costs ~40us/call dispatch
  overhead through the PJRT relay), bf16 output halves the out DMA.

Self-contained: hardcodes shapes B=2, T=2048, D=1024, H=16, HD=64.
"""
import os
import sys
import math
from contextlib import ExitStack

for _p in ("/opt/trn_rl_repo", "/root/.axon_site/_ro/trn_rl_repo"):
    if os.path.isdir(_p) and _p not in sys.path:
        sys.path.append(_p)

import numpy as np

import concourse.bass as bass
import concourse.bacc as bacc
import concourse.mybir as mybir
import concourse.tile as tile
from concourse.bass_utils import run_bass_kernel_spmd

B, T, D = 2, 2048, 1024
H, HD = 16, 64
EPS = 1e-6
NCORES = 8
HPC = 4          # heads per core
F = HPC * HD     # 256 features per core per q/k/v
NTB = T // 128   # 16 t-blocks
NIB = T // 512   # 4 i-blocks
MASKVAL = -200.0
F32 = mybir.dt.float32
BF = mybir.dt.bfloat16
AF = mybir.ActivationFunctionType
ALU = mybir.AluOpType
AX = mybir.AxisListType

# wb column layout (bf16): wq | wp | cg | sg | ident | ones | maskT
WQ0, WP0, CG0, SG0, ID0, ON0, MT0, WBC = (
    0, 6144, 8192, 10240, 12288, 12416, 12544, 12672)


def build_program(exp_bias: float, has_bias: bool = True):
    """One SPMD program; per-core behavior differs only via input data."""
    nc = bacc.Bacc("TRN2", target_bir_lowering=False)
    xt8 = nc.declare_dram_parameter("xt8", [128, 8 * T], BF, isOutput=False)
    wb = nc.declare_dram_parameter("wb", [128, WBC], BF, isOutput=False)
    bqkv = (nc.declare_dram_parameter("bqkv", [1, 3 * F], F32, isOutput=False)
            if has_bias else None)
    outp = nc.declare_dram_parameter("outp", [T, D], BF, isOutput=True)

    with tile.TileContext(nc) as tc, ExitStack() as ctx:
        cpool = ctx.enter_context(tc.tile_pool(name="consts", bufs=1))
        workq = ctx.enter_context(tc.tile_pool(name="workq", bufs=2))
        qrp = ctx.enter_context(tc.tile_pool(name="qrp", bufs=3))
        smallp = ctx.enter_context(tc.tile_pool(name="smallp", bufs=2))
        epool = ctx.enter_context(tc.tile_pool(name="epool", bufs=3))
        rvp = ctx.enter_context(tc.tile_pool(name="rvp", bufs=2))
        stp = ctx.enter_context(tc.tile_pool(name="stp", bufs=2))
        # PSUM: 8 banks.  ps1: 2 x 1-bank (pq/pv in phase A; pj halves and the
        # denominator-broadcast tiles in B/C); ps2: 2 x 2-bank (transposes in
        # A, S^T tiles in B); pop: 2 x 1-bank (per-head PV accumulators).
        ps1 = ctx.enter_context(tc.tile_pool(name="ps1", bufs=2, space="PSUM"))
        ps2 = ctx.enter_context(tc.tile_pool(name="ps2", bufs=2, space="PSUM"))
        pop = ctx.enter_context(tc.tile_pool(name="pop", bufs=2, space="PSUM"))

        # ---- persistent SBUF ----
        xt_sb = cpool.tile([128, 8, T], BF, tag="xt")
        wq_sb = cpool.tile([128, 8, 3 * F], BF, tag="wq")
        csg = cpool.tile([128, 2, NTB, 128], BF, tag="csg")  # cos|sin rope
        wp_sb = cpool.tile([128, 2, D], BF, tag="wp")
        cm_sb = cpool.tile([128, 3, 128], BF, tag="cm")      # ident|ones|maskT
        eps_b = cpool.tile([128, 1], F32, tag="epsb")
        ebias_b = cpool.tile([128, 1], F32, tag="ebiasb")
        qkt = cpool.tile([128, 4, T], BF, tag="qkt")     # [c(2 heads), {q,k}, t]
        v4 = cpool.tile([128, NTB, HPC, HD + 1], BF, tag="v4")  # V|1
        ont = cpool.tile([128, 2, T], BF, tag="ont")     # normalized O^T chunks
        bb = cpool.tile([128, 3 * F], F32, tag="bb") if has_bias else None
        id_sb = cm_sb[:, 0, :]
        onesP = cm_sb[:, 1, :]
        mt_sb = cm_sb[:, 2, :]

        # ---- prologue DMAs: first wave (wq + first xt chunks + rope/consts,
        # spread over three queues) so QKV starts ~7us in; rest trails ----
        wqv = wb[:, WQ0:WP0].rearrange("p (k f) -> p k f", k=8)
        xtv = xt8.rearrange("p (k t) -> p k t", k=8)
        csgv = wb[:, CG0:ID0].rearrange("p (g t c) -> p g t c", g=2, t=NTB)
        nc.scalar.dma_start(out=wq_sb[:, 0:2, :], in_=wqv[:, 0:2, :])
        nc.sync.dma_start(out=xt_sb[:, :, 0:128], in_=xtv[:, :, 0:128])
        nc.gpsimd.dma_start(out=wq_sb[:, 6:8, :], in_=wqv[:, 6:8, :])
        nc.scalar.dma_start(out=wq_sb[:, 2:4, :], in_=wqv[:, 2:4, :])
        nc.sync.dma_start(out=wq_sb[:, 4:6, :], in_=wqv[:, 4:6, :])
        nc.gpsimd.dma_start(out=csg[:, :, 0:4, :], in_=csgv[:, :, 0:4, :])
        nc.sync.dma_start(out=xt_sb[:, :, 128:512], in_=xtv[:, :, 128:512])
        nc.gpsimd.dma_start(out=csg[:, :, 4:16, :], in_=csgv[:, :, 4:16, :])
        nc.scalar.dma_start(out=cm_sb, in_=wb[:, ID0:WBC].rearrange(
            "p (g c) -> p g c", g=3))
        nc.vector.memset(eps_b, EPS)
        nc.vector.memset(ebias_b, float(exp_bias))
        # V ones-columns (softmax denominator trick), written once
        nc.scalar.copy(
            out=v4[:, :, :, HD:HD + 1],
            in_=onesP[:, 0:NTB * HPC].rearrange("p (t h) -> p t h", t=NTB).unsqueeze(3),
        )
        if has_bias:
            bq_in = stp.tile([1, 3 * F], F32, tag="bqin")
            nc.gpsimd.dma_start(out=bq_in, in_=bqkv[:, :])
            nc.gpsimd.partition_broadcast(bb, bq_in)
        # second wave: xt tail + c_proj weights
        nc.sync.dma_start(out=xt_sb[:, :, 512:1024], in_=xtv[:, :, 512:1024])
        nc.sync.dma_start(out=xt_sb[:, :, 1024:1536], in_=xtv[:, :, 1024:1536])
        nc.sync.dma_start(out=xt_sb[:, :, 1536:2048], in_=xtv[:, :, 1536:2048])
        nc.gpsimd.dma_start(
            out=wp_sb, in_=wb[:, WP0:CG0].rearrange("p (k f) -> p k f", k=2))

        # ---- phase A: QKV projection + rmsnorm + rope; transposes trail ----
        def emit_qkv(tb):
            ts = slice(tb * 128, (tb + 1) * 128)
            pq = ps1.tile([128, 512], F32, tag="ps1", name=f"pq{tb}")
            pv = ps1.tile([128, 256], F32, tag="ps1", name=f"pv{tb}")
            # pq completes before the pv loop: its PSUM slot frees while the
            # pv matmuls still run, so the next t-block's pq isn't gated
            for kd in range(8):
                nc.tensor.matmul(pq, xt_sb[:, kd, ts], wq_sb[:, kd, 0:512],
                                 start=(kd == 0), stop=(kd == 7))
            for kd in range(8):
                nc.tensor.matmul(pv, xt_sb[:, kd, ts], wq_sb[:, kd, 512:768],
                                 start=(kd == 0), stop=(kd == 7))
            # stage q|k to SBUF in bf16 (Act copy: Act has slack in the
            # QKV-heavy stretches; DVE is the busier engine there)
            pqs = workq.tile([128, 512], BF, tag="pqs", name=f"pqs{tb}")
            if has_bias:
                pqf = workq.tile([128, 512], F32, tag="pqf", name=f"pqf{tb}")
                nc.scalar.copy(out=pqf, in_=pq)
                nc.vector.tensor_tensor(pqs, pqf, bb[:, 0:512], ALU.add)
            else:
                nc.scalar.copy(out=pqs, in_=pq)
            # V -> SBUF [t, head, c] (bf16); DVE in the Act-bound middle
            # stretch, Act in the DVE-bound last t-blocks
            pvv = pv.rearrange("p (h c) -> p h c", h=HPC)
            if has_bias:
                bbv = bb[:, 512:768].rearrange("p (h c) -> p h c", h=HPC)
                nc.vector.tensor_tensor(v4[:, tb, :, 0:HD], pvv, bbv, ALU.add)
            elif 5 <= tb <= 10:
                nc.vector.tensor_copy(v4[:, tb, :, 0:HD], pvv)
            else:
                nc.scalar.copy(out=v4[:, tb, :, 0:HD], in_=pvv)
            # rmsnorm stats (Pool does the square; DVE reduces; Act rsqrts)
            sq = workq.tile([128, 512], BF, tag="sq", name=f"sq{tb}")
            nc.gpsimd.tensor_mul(sq, pqs, pqs)
            var = smallp.tile([128, 8], F32, tag="var", name=f"var{tb}")
            nc.vector.tensor_reduce(
                var, sq.rearrange("p (h c) -> p h c", h=8), AX.X, ALU.add
            )
            rstd_s = smallp.tile([128, 8], F32, tag="rstds", name=f"rstds{tb}")
            nc.scalar.activation(rstd_s, var, AF.Sqrt, scale=1.0 / HD,
                                 bias=eps_b[:, :])
            rstd = smallp.tile([128, 8], BF, tag="rstd", name=f"rstd{tb}")
            with nc.allow_low_precision(reason="bf16 rmsnorm scale"):
                nc.vector.reciprocal(rstd, rstd_s)
            # qn = q * rstd (per-head broadcast), all-bf16 => DVE 2x mode
            qn = workq.tile([128, 512], BF, tag="qn", name=f"qn{tb}")
            nc.vector.tensor_tensor(
                qn.rearrange("p (h c) -> p h c", h=8),
                pqs.rearrange("p (h c) -> p h c", h=8),
                rstd.unsqueeze(2).broadcast_to((128, 8, HD)),
                ALU.mult,
            )
            # rope: qr = qn*CG + shift(qn)*SG   (bf16 throughout)
            qn4 = qn.rearrange("p (g h c) -> p g h c", g=2, h=HPC)
            cgs = csg[:, 0, tb, :].rearrange("p (g c) -> p g c", g=2)
            sgs = csg[:, 1, tb, :].rearrange("p (g c) -> p g c", g=2)
            m1 = workq.tile([128, 512], BF, tag="m1", name=f"m1_{tb}")
            m1v = m1.rearrange("p (g h c) -> p g h c", g=2, h=HPC)
            nc.vector.tensor_tensor(
                m1v, qn4, cgs.unsqueeze(2).broadcast_to((128, 2, HPC, HD)), ALU.mult
            )
            m2 = workq.tile([128, 512], BF, tag="m2", name=f"m2_{tb}")
            m2v = m2.rearrange("p (g h c) -> p g h c", g=2, h=HPC)
            nc.vector.tensor_tensor(
                m2v[:, :, :, 0:32],
                qn4[:, :, :, 32:64],
                sgs[:, :, 0:32].unsqueeze(2).broadcast_to((128, 2, HPC, 32)),
                ALU.mult,
            )
            nc.vector.tensor_tensor(
                m2v[:, :, :, 32:64],
                qn4[:, :, :, 0:32],
                sgs[:, :, 32:64].unsqueeze(2).broadcast_to((128, 2, HPC, 32)),
                ALU.mult,
            )
            qr = qrp.tile([128, 512], BF, tag="qr", name=f"qr{tb}")
            nc.vector.tensor_add(qr, m1, m2)
            return qr

        def emit_tr(tb, qr):
            ts = slice(tb * 128, (tb + 1) * 128)
            tr = ps2.tile([128, 4, 128], BF, tag="ps2", name=f"tr{tb}")
            for cc in range(4):
                nc.tensor.transpose(tr[:, cc, :], qr[:, cc * 128:(cc + 1) * 128],
                                    id_sb)
            nc.vector.tensor_copy(qkt[:, :, ts], tr[:, 0:4, :])

        # ---- phase B: attention (jc-pipelined), with C woven in ----
        def emit_sp(hp, ib, jc):
            s = 128 * max(0, jc - 4 * ib)
            diag = jc >= 4 * ib
            isl = slice(ib * 512 + s, (ib + 1) * 512)
            jsl = slice(jc * 128, (jc + 1) * 128)
            sp = ps2.tile([128, 2, 512], F32, tag="ps2", name=f"sp{hp}_{ib}_{jc}")
            nc.tensor.matmul(
                sp[:, 0, s:512], qkt[0:64, 2 + hp, jsl], qkt[0:64, hp, isl],
                start=True, stop=not diag, tile_position=(0, 0),
            )
            nc.tensor.matmul(
                sp[:, 1, s:512], qkt[64:128, 2 + hp, jsl], qkt[64:128, hp, isl],
                start=True, stop=not diag, tile_position=(64, 0),
            )
            if diag:
                # additive causal mask: sp[j, i] += -200 * [j > i] on the
                # diagonal 128x128 block (stationary maskT, moving identity)
                for h in range(2):
                    nc.tensor.matmul(
                        sp[:, h, s:s + 128], mt_sb, id_sb,
                        start=False, stop=True, tile_position=(0, 0),
                        skip_group_check=True,
                    )
            return sp

        def emit_fin(hp, ib, jc, sp, po, njc):
            s = 128 * max(0, jc - 4 * ib)
            e = epool.tile([128, 2, 512], BF, tag="e", name=f"e{hp}_{ib}_{jc}")
            nc.scalar.activation(
                e[:, :, s:512], sp[:, :, s:512], AF.Exp,
                scale=1.0 / math.sqrt(HD), bias=ebias_b[:, :],
            )
            first, last = (jc == 0), (jc == njc - 1)
            for h in range(2):
                head = hp * 2 + h
                nc.tensor.matmul(
                    po[h][:, s:512], v4[:, jc, head, :], e[:, h, s:512],
                    start=first, stop=last,
                )

        def emit_norm(hp, ib, po):
            isl = slice(ib * 512, (ib + 1) * 512)
            rv = rvp.tile([128, 1024], BF, tag="rv", name=f"rv{hp}_{ib}")
            for h in range(2):
                with nc.allow_low_precision(reason="bf16 softmax denom"):
                    nc.vector.reciprocal(
                        rv[64:65, h * 512:(h + 1) * 512], po[h][64:65, :]
                    )
            pos = [rvp.tile([65, 512], F32, tag="pos", name=f"pos{hp}_{ib}_{h}")
                   for h in range(2)]
            nc.scalar.copy(out=pos[0], in_=po[0])
            nc.vector.tensor_copy(pos[1], po[1])
            pb = [pop.tile([64, 512], F32, tag="po", name=f"pb{hp}_{ib}_{h}")
                  for h in range(2)]
            for h in range(2):
                nc.tensor.matmul(
                    pb[h], onesP[64:65, 0:64], rv[64:65, h * 512:(h + 1) * 512],
                    start=True, stop=True,
                )
            nc.vector.tensor_mul(ont[0:64, hp, isl], pos[0][0:64, :], pb[0])
            stage = stp.tile([64, 512], BF, tag="stage", name=f"st{hp}_{ib}")
            nc.vector.tensor_mul(stage, pos[1][0:64, :], pb[1])
            eng = nc.scalar if (ib == NIB - 1 and hp == 1) else nc.sync
            eng.dma_start(out=ont[64:128, hp, isl], in_=stage)

        def emit_cfull(tb, tail=False):
            # whole 128-token output block: the two 512-col halves share their
            # stationary ont blocks (hp-major matmul order)
            ts = slice(tb * 128, (tb + 1) * 128)
            pj = [ps1.tile([128, 512], F32, tag="ps1", name=f"pj{tb}_{nh}")
                  for nh in range(2)]
            for hp in range(2):
                for nh in range(2):
                    nc.tensor.matmul(
                        pj[nh], ont[:, hp, ts], wp_sb[:, hp, nh * 512:(nh + 1) * 512],
                        start=(hp == 0), stop=(hp == 1), skip_group_check=True,
                    )
            ob = stp.tile([128, D], BF, tag="ob", name=f"ob{tb}")
            if tail:
                nc.vector.tensor_copy(ob[:, 0:512], pj[0])
                nc.scalar.copy(out=ob[:, 512:1024], in_=pj[1])
                nc.scalar.dma_start(out=outp[ts, :], in_=ob)
            else:
                nc.vector.tensor_copy(ob[:, 0:512], pj[0])
                nc.vector.tensor_copy(ob[:, 512:1024], pj[1])
                nc.sync.dma_start(out=outp[ts, :], in_=ob)

        # ---- phase A: QKV projection pipeline; transposes trail two blocks.
        # i-block 0's attention (short pipelines, Act otherwise idle here) is
        # woven one step per t-block into A's back half; the last two
        # transposes (only needed by i-block 3) are deferred into phase B so
        # they don't head-of-line-block the PE queue while the DVE rope chain
        # for the last t-blocks drains. ----
        qrs = {}
        ib0 = {}

        def ib0_unit(i):
            hp, u = divmod(i, 5)
            if u == 0:
                ib0['po', hp] = [
                    pop.tile([65, 512], F32, tag="po", name=f"po{hp}_0_{h}")
                    for h in range(2)]
                ib0['sp', hp, 0] = emit_sp(hp, 0, 0)
            elif u <= 3:
                sp_cur = emit_sp(hp, 0, u)
                emit_fin(hp, 0, u - 1, ib0.pop(('sp', hp, u - 1)),
                         ib0['po', hp], 4)
                ib0['sp', hp, u] = sp_cur
            else:
                emit_fin(hp, 0, 3, ib0.pop(('sp', hp, 3)), ib0['po', hp], 4)
                emit_norm(hp, 0, ib0.pop(('po', hp)))

        for tb in range(NTB):
            if tb >= 2:
                emit_tr(tb - 2, qrs.pop(tb - 2))
            qrs[tb] = emit_qkv(tb)
            if tb >= 6:
                ib0_unit(tb - 6)
        deferred_tr = [NTB - 2, NTB - 1]

        # ---- phase B: attention (jc-pipelined), with C woven in ----
        for ib in range(1, NIB):
            njc = 4 * ib + 4
            blocks = list(range(4 * (ib - 1), 4 * ib)) if ib > 0 else []
            bidx = 0
            for hp in range(2):
                po = [pop.tile([65, 512], F32, tag="po", name=f"po{hp}_{ib}_{h}")
                      for h in range(2)]
                sp_prev = emit_sp(hp, ib, 0)
                if deferred_tr:
                    emit_tr(deferred_tr[0], qrs.pop(deferred_tr.pop(0)))
                for jc in range(1, njc):
                    sp_cur = emit_sp(hp, ib, jc)
                    emit_fin(hp, ib, jc - 1, sp_prev, po, njc)
                    sp_prev = sp_cur
                    if jc % 3 == 1 and bidx < len(blocks):
                        emit_cfull(blocks[bidx])
                        bidx += 1
                emit_fin(hp, ib, njc - 1, sp_prev, po, njc)
                emit_norm(hp, ib, po)
            while bidx < len(blocks):
                emit_cfull(blocks[bidx])
                bidx += 1
        for tb in range(12, 16):
            emit_cfull(tb, tail=True)

    nc.compile()
    return nc


def host_inputs(x, w_attn, b_attn, w_proj, g_q, g_k, rope_cos, rope_sin):
    """Per-core input maps + exp bias."""
    import ml_dtypes
    bf16 = ml_dtypes.bfloat16
    x = np.asarray(x, dtype=np.float32)
    w_attn = np.asarray(w_attn, dtype=np.float32)
    b_attn = np.asarray(b_attn, dtype=np.float32)
    w_proj = np.asarray(w_proj, dtype=np.float32)
    g_q = np.asarray(g_q, dtype=np.float32)
    g_k = np.asarray(g_k, dtype=np.float32)
    rope_cos = np.asarray(rope_cos, dtype=np.float32)
    rope_sin = np.asarray(rope_sin, dtype=np.float32)

    # |s| <= 8 * max|g_q| * max|g_k| after RMSNorm; subtract for exp safety
    bound = 8.0 * max(1e-6, float(np.abs(g_q).max())) * max(
        1e-6, float(np.abs(g_k).max())
    )
    exp_bias = -bound

    # rope tables with gains folded in; shifted-sign sin for rotate_half
    def sg_of(g):
        sgn = np.where(np.arange(HD) < HD // 2, -1.0, 1.0).astype(np.float32)
        gperm = np.roll(g, HD // 2)  # g[(c+32)%64]
        return rope_sin * (sgn * gperm)[None, :]  # [T, HD]

    cgq = rope_cos * g_q[None, :]
    cgk = rope_cos * g_k[None, :]
    sgq = sg_of(g_q)
    sgk = sg_of(g_k)

    def arrange_rope(a_q, a_k):
        # [T, HD] x2 -> [128, NTB*128] with [p, tb, {q:64 | k:64}]
        aq = a_q.reshape(NTB, 128, HD).transpose(1, 0, 2)
        ak = a_k.reshape(NTB, 128, HD).transpose(1, 0, 2)
        return np.ascontiguousarray(
            np.concatenate([aq, ak], axis=2).reshape(128, NTB * 128)
        ).astype(np.float32)

    cg_arr = arrange_rope(cgq, cgk)
    sg_arr = arrange_rope(sgq, sgk)

    ident = np.eye(128, dtype=np.float32)
    ones = np.ones((128, 128), dtype=np.float32)
    # maskT[p, j] = MASKVAL for j > p: the mask matmul (stationary maskT,
    # moving identity) adds MASKVAL at sp[j, i] for key j > query i
    maskT = MASKVAL * np.triu(np.ones((128, 128), np.float32), 1)

    in_maps = []
    for c in range(NCORES):
        b, hg = divmod(c, 4)
        f0 = hg * F
        rows = np.concatenate([
            np.arange(f0, f0 + F),
            D + np.arange(f0, f0 + F),
            2 * D + np.arange(f0, f0 + F),
        ])
        w = w_attn[rows]                      # [768, 1024]
        wqkvT = np.ascontiguousarray(w.T)     # [1024, 768]
        wqkv8 = wqkvT.reshape(8, 128, 3 * F).transpose(1, 0, 2).reshape(128, 8 * 3 * F)
        wpT = np.ascontiguousarray(w_proj[:, f0:f0 + F].T)  # [256, 1024]
        wp2 = wpT.reshape(2, 128, D).transpose(1, 0, 2).reshape(128, 2 * D)
        wb = np.concatenate(
            [wqkv8, wp2, cg_arr, sg_arr, ident, ones, maskT], axis=1
        ).astype(bf16)
        assert wb.shape == (128, WBC)
        xtT = np.ascontiguousarray(x[b].T)    # [1024, 2048]
        xt8 = np.ascontiguousarray(
            xtT.reshape(8, 128, T).transpose(1, 0, 2).reshape(128, 8 * T)
        ).astype(bf16)
        m = {"xt8": xt8, "wb": np.ascontiguousarray(wb)}
        if bool(np.any(b_attn)):
            m["bqkv"] = np.ascontiguousarray(b_attn[rows].reshape(1, 3 * F))
        in_maps.append(m)
    return in_maps, exp_bias


_CACHE = {}


def kernel(x, w_attn, b_attn, w_proj, b_proj, g_q, g_k, rope_cos, rope_sin):
    in_maps, exp_bias = host_inputs(
        x, w_attn, b_attn, w_proj, g_q, g_k, rope_cos, rope_sin
    )
    has_bias = bool(np.any(np.asarray(b_attn)))
    key = (float(exp_bias), has_bias)
    if key not in _CACHE:
        _CACHE[key] = build_program(exp_bias, has_bias)
    nc = _CACHE[key]
    res = run_bass_kernel_spmd(nc, in_maps, list(range(NCORES)))
    out = np.zeros((B, T, D), dtype=np.float32)
    for c in range(NCORES):
        out[c // 4] += np.asarray(res.results[c]["outp"], dtype=np.float32)
    out += np.asarray(b_proj, dtype=np.float32)[None, None, :]
    return out


# revision 4
# speedup vs baseline: 3.5601x; 3.5601x over previous
"""Trainium2 Bass kernel for CausalSelfAttention (QK-RMSNorm + RoPE).

Sharding: 8 cores = 2 batches x 4 head-groups (4 heads each).
Each core computes QKV projection for its heads, attention, and a partial
output projection (row-parallel c_proj); host sums the 4 partials per batch
and adds b_proj.

v2 changes vs the f32r baseline:
- attention core in bf16 end to end (q/k^T, exp(S), V, O^T, c_proj weights):
  bf16 moving operands run the PE at 1 cyc/row with no small-tile penalty,
  and bf16 transposes are 1.0 cyc/row instead of 1.5.
- causal triangle applied as an additive -200 mask accumulated into the S
  PSUM tile by a pair of tiny PE matmuls (stationary = upper-triangular
  -200 matrix, moving = identity), replacing the Pool multiply on exp(S).
- QKV projection interleaves the q|k and v matmuls per contraction chunk so
  consecutive matmuls share the stationary x-block (half the Ldweights).
- output projection emits whole t-blocks (both 512-col halves) so the two
  halves share their stationary O^T blocks.
- softmax normalization multiplies PSUM operands directly (no staging copy).
- all constants packed into one bf16 DRAM parameter (wb) next to x^T: 2
  input buffers instead of 9 (each extra buffer costs ~40us/call dispatch
  overhead through the PJRT relay), bf16 output halves the out DMA.

Self-contained: hardcodes shapes B=2, T=2048, D=1024, H=16, HD=64.
"""
import os
import sys
import math
from contextlib import ExitStack

for _p in ("/opt/trn_rl_repo", "/root/.axon_site/_ro/trn_rl_repo"):
    if os.path.isdir(_p) and _p not in sys.path:
        sys.path.append(_p)

import numpy as np

import concourse.bass as bass
import concourse.bacc as bacc
import concourse.mybir as mybir
import concourse.tile as tile
from concourse.bass_utils import run_bass_kernel_spmd

B, T, D = 2, 2048, 1024
H, HD = 16, 64
EPS = 1e-6
NCORES = 8
HPC = 4          # heads per core
F = HPC * HD     # 256 features per core per q/k/v
NTB = T // 128   # 16 t-blocks
NIB = T // 512   # 4 i-blocks
MASKVAL = -200.0
F32 = mybir.dt.float32
BF = mybir.dt.bfloat16
AF = mybir.ActivationFunctionType
ALU = mybir.AluOpType
AX = mybir.AxisListType

# wb column layout (bf16): wq | wp | cg | sg | ident | ones | maskT
WQ0, WP0, CG0, SG0, ID0, ON0, MT0, WBC = (
    0, 6144, 8192, 10240, 12288, 12416, 12544, 12672)


def build_program(exp_bias: float, has_bias: bool = True):
    """One SPMD program; per-core behavior differs only via input data."""
    nc = bacc.Bacc("TRN2", target_bir_lowering=False)
    xt8 = nc.declare_dram_parameter("xt8", [128, 8 * T], BF, isOutput=False)
    wb = nc.declare_dram_parameter("wb", [128, WBC], BF, isOutput=False)
    bqkv = (nc.declare_dram_parameter("bqkv", [1, 3 * F], F32, isOutput=False)
            if has_bias else None)
    outp = nc.declare_dram_parameter("outp", [T, D], BF, isOutput=True)

    with tile.TileContext(nc) as tc, ExitStack() as ctx:
        cpool = ctx.enter_context(tc.tile_pool(name="consts", bufs=1))
        workq = ctx.enter_context(tc.tile_pool(name="workq", bufs=2))
        qrp = ctx.enter_context(tc.tile_pool(name="qrp", bufs=3))
        smallp = ctx.enter_context(tc.tile_pool(name="smallp", bufs=2))
        epool = ctx.enter_context(tc.tile_pool(name="epool", bufs=3))
        rvp = ctx.enter_context(tc.tile_pool(name="rvp", bufs=2))
        stp = ctx.enter_context(tc.tile_pool(name="stp", bufs=2))
        # PSUM: 8 banks.  ps1: 2 x 1-bank (pq/pv in phase A; pj halves and the
        # denominator-broadcast tiles in B/C); ps2: 2 x 2-bank (transposes in
        # A, S^T tiles in B); pop: 2 x 1-bank (per-head PV accumulators).
        ps1 = ctx.enter_context(tc.tile_pool(name="ps1", bufs=2, space="PSUM"))
        ps2 = ctx.enter_context(tc.tile_pool(name="ps2", bufs=2, space="PSUM"))
        pop = ctx.enter_context(tc.tile_pool(name="pop", bufs=2, space="PSUM"))

        # ---- persistent SBUF ----
        xt_sb = cpool.tile([128, 8, T], BF, tag="xt")
        wq_sb = cpool.tile([128, 8, 3 * F], BF, tag="wq")
        csg = cpool.tile([128, 2, NTB, 128], BF, tag="csg")  # cos|sin rope
        wp_sb = cpool.tile([128, 2, D], BF, tag="wp")
        cm_sb = cpool.tile([128, 3, 128], BF, tag="cm")      # ident|ones|maskT
        eps_b = cpool.tile([128, 1], F32, tag="epsb")
        ebias_b = cpool.tile([128, 1], F32, tag="ebiasb")
        qkt = cpool.tile([128, 4, T], BF, tag="qkt")     # [c(2 heads), {q,k}, t]
        v4 = cpool.tile([128, NTB, HPC, HD + 1], BF, tag="v4")  # V|1
        ont = cpool.tile([128, 2, T], BF, tag="ont")     # normalized O^T chunks
        bb = cpool.tile([128, 3 * F], F32, tag="bb") if has_bias else None
        id_sb = cm_sb[:, 0, :]
        onesP = cm_sb[:, 1, :]
        mt_sb = cm_sb[:, 2, :]

        # ---- prologue DMAs: first wave (wq + first xt chunks + rope/consts,
        # spread over three queues) so QKV starts ~7us in; rest trails ----
        wqv = wb[:, WQ0:WP0].rearrange("p (k f) -> p k f", k=8)
        xtv = xt8.rearrange("p (k t) -> p k t", k=8)
        csgv = wb[:, CG0:ID0].rearrange("p (g t c) -> p g t c", g=2, t=NTB)
        nc.scalar.dma_start(out=wq_sb[:, 0:2, :], in_=wqv[:, 0:2, :])
        nc.sync.dma_start(out=xt_sb[:, :, 0:128], in_=xtv[:, :, 0:128])
        nc.gpsimd.dma_start(out=wq_sb[:, 6:8, :], in_=wqv[:, 6:8, :])
        nc.scalar.dma_start(out=wq_sb[:, 2:4, :], in_=wqv[:, 2:4, :])
        nc.sync.dma_start(out=wq_sb[:, 4:6, :], in_=wqv[:, 4:6, :])
        nc.gpsimd.dma_start(out=csg[:, :, 0:4, :], in_=csgv[:, :, 0:4, :])
        nc.sync.dma_start(out=xt_sb[:, :, 128:512], in_=xtv[:, :, 128:512])
        nc.gpsimd.dma_start(out=csg[:, :, 4:16, :], in_=csgv[:, :, 4:16, :])
        nc.scalar.dma_start(out=cm_sb, in_=wb[:, ID0:WBC].rearrange(
            "p (g c) -> p g c", g=3))
        nc.vector.memset(eps_b, EPS)
        nc.vector.memset(ebias_b, float(exp_bias))
        # V ones-columns (softmax denominator trick), written once
        nc.scalar.copy(
            out=v4[:, :, :, HD:HD + 1],
            in_=onesP[:, 0:NTB * HPC].rearrange("p (t h) -> p t h", t=NTB).unsqueeze(3),
        )
        if has_bias:
            bq_in = stp.tile([1, 3 * F], F32, tag="bqin")
            nc.gpsimd.dma_start(out=bq_in, in_=bqkv[:, :])
            nc.gpsimd.partition_broadcast(bb, bq_in)
        # second wave: xt tail + c_proj weights
        nc.sync.dma_start(out=xt_sb[:, :, 512:1024], in_=xtv[:, :, 512:1024])
        nc.sync.dma_start(out=xt_sb[:, :, 1024:1536], in_=xtv[:, :, 1024:1536])
        nc.sync.dma_start(out=xt_sb[:, :, 1536:2048], in_=xtv[:, :, 1536:2048])
        nc.gpsimd.dma_start(
            out=wp_sb, in_=wb[:, WP0:CG0].rearrange("p (k f) -> p k f", k=2))

        # ---- phase A: QKV projection + rmsnorm + rope; transposes trail ----
        def emit_qkv(tb):
            ts = slice(tb * 128, (tb + 1) * 128)
            pq = ps1.tile([128, 512], F32, tag="ps1", name=f"pq{tb}")
            pv = ps1.tile([128, 256], F32, tag="ps1", name=f"pv{tb}")
            # pq completes before the pv loop: its PSUM slot frees while the
            # pv matmuls still run, so the next t-block's pq isn't gated
            for kd in range(8):
                nc.tensor.matmul(pq, xt_sb[:, kd, ts], wq_sb[:, kd, 0:512],
                                 start=(kd == 0), stop=(kd == 7))
            for kd in range(8):
                nc.tensor.matmul(pv, xt_sb[:, kd, ts], wq_sb[:, kd, 512:768],
                                 start=(kd == 0), stop=(kd == 7))
            # stage q|k to SBUF in bf16 (Act copy: Act has slack in the
            # QKV-heavy stretches; DVE is the busier engine there)
            pqs = workq.tile([128, 512], BF, tag="pqs", name=f"pqs{tb}")
            if has_bias:
                pqf = workq.tile([128, 512], F32, tag="pqf", name=f"pqf{tb}")
                nc.scalar.copy(out=pqf, in_=pq)
                nc.vector.tensor_tensor(pqs, pqf, bb[:, 0:512], ALU.add)
            else:
                nc.scalar.copy(out=pqs, in_=pq)
            # V -> SBUF [t, head, c] (bf16); DVE in the Act-bound middle
            # stretch, Act in the DVE-bound last t-blocks
            pvv = pv.rearrange("p (h c) -> p h c", h=HPC)
            if has_bias:
                bbv = bb[:, 512:768].rearrange("p (h c) -> p h c", h=HPC)
                nc.vector.tensor_tensor(v4[:, tb, :, 0:HD], pvv, bbv, ALU.add)
            elif 5 <= tb <= 10:
                nc.vector.tensor_copy(v4[:, tb, :, 0:HD], pvv)
            else:
                nc.scalar.copy(out=v4[:, tb, :, 0:HD], in_=pvv)
            # rmsnorm stats (Pool does the square; DVE reduces; Act rsqrts)
            sq = workq.tile([128, 512], BF, tag="sq", name=f"sq{tb}")
            nc.gpsimd.tensor_mul(sq, pqs, pqs)
            var = smallp.tile([128, 8], F32, tag="var", name=f"var{tb}")
            nc.vector.tensor_reduce(
                var, sq.rearrange("p (h c) -> p h c", h=8), AX.X, ALU.add
            )
            rstd_s = smallp.tile([128, 8], F32, tag="rstds", name=f"rstds{tb}")
            nc.scalar.activation(rstd_s, var, AF.Sqrt, scale=1.0 / HD,
                                 bias=eps_b[:, :])
            rstd = smallp.tile([128, 8], BF, tag="rstd", name=f"rstd{tb}")
            with nc.allow_low_precision(reason="bf16 rmsnorm scale"):
                nc.vector.reciprocal(rstd, rstd_s)
            # qn = q * rstd (per-head broadcast), all-bf16 => DVE 2x mode
            qn = workq.tile([128, 512], BF, tag="qn", name=f"qn{tb}")
            nc.vector.tensor_tensor(
                qn.rearrange("p (h c) -> p h c", h=8),
                pqs.rearrange("p (h c) -> p h c", h=8),
                rstd.unsqueeze(2).broadcast_to((128, 8, HD)),
                ALU.mult,
            )
            # rope: qr = qn*CG + shift(qn)*SG   (bf16 throughout)
            qn4 = qn.rearrange("p (g h c) -> p g h c", g=2, h=HPC)
            cgs = csg[:, 0, tb, :].rearrange("p (g c) -> p g c", g=2)
            sgs = csg[:, 1, tb, :].rearrange("p (g c) -> p g c", g=2)
            m1 = workq.tile([128, 512], BF, tag="m1", name=f"m1_{tb}")
            m1v = m1.rearrange("p (g h c) -> p g h c", g=2, h=HPC)
            nc.vector.tensor_tensor(
                m1v, qn4, cgs.unsqueeze(2).broadcast_to((128, 2, HPC, HD)), ALU.mult
            )
            m2 = workq.tile([128, 512], BF, tag="m2", name=f"m2_{tb}")
            m2v = m2.rearrange("p (g h c) -> p g h c", g=2, h=HPC)
            nc.vector.tensor_tensor(
                m2v[:, :, :, 0:32],
                qn4[:, :, :, 32:64],
                sgs[:, :, 0:32].unsqueeze(2).broadcast_to((128, 2, HPC, 32)),
                ALU.mult,
            )
            nc.vector.tensor_tensor(
                m2v[:, :, :, 32:64],
                qn4[:, :, :, 0:32],
                sgs[:, :, 32:64].unsqueeze(2).broadcast_to((128, 2, HPC, 32)),
                ALU.mult,
            )
            qr = qrp.tile([128, 512], BF, tag="qr", name=f"qr{tb}")
            nc.vector.tensor_add(qr, m1, m2)
            return qr

        def emit_tr(tb, qr):
            ts = slice(tb * 128, (tb + 1) * 128)
            tr = ps2.tile([128, 4, 128], BF, tag="ps2", name=f"tr{tb}")
            for cc in range(4):
                nc.tensor.transpose(tr[:, cc, :], qr[:, cc * 128:(cc + 1) * 128],
                                    id_sb)
            nc.vector.tensor_copy(qkt[:, :, ts], tr[:, 0:4, :])

        # ---- phase B: attention (jc-pipelined), with C woven in ----
        def emit_sp(hp, ib, jc):
            s = 128 * max(0, jc - 4 * ib)
            diag = jc >= 4 * ib
            isl = slice(ib * 512 + s, (ib + 1) * 512)
            jsl = slice(jc * 128, (jc + 1) * 128)
            sp = ps2.tile([128, 2, 512], F32, tag="ps2", name=f"sp{hp}_{ib}_{jc}")
            nc.tensor.matmul(
                sp[:, 0, s:512], qkt[0:64, 2 + hp, jsl], qkt[0:64, hp, isl],
                start=True, stop=not diag, tile_position=(0, 0),
            )
            nc.tensor.matmul(
                sp[:, 1, s:512], qkt[64:128, 2 + hp, jsl], qkt[64:128, hp, isl],
                start=True, stop=not diag, tile_position=(64, 0),
            )
            if diag:
                # additive causal mask: sp[j, i] += -200 * [j > i] on the
                # diagonal 128x128 block (stationary maskT, moving identity)
                for h in range(2):
                    nc.tensor.matmul(
                        sp[:, h, s:s + 128], mt_sb, id_sb,
                        start=False, stop=True, tile_position=(0, 0),
                        skip_group_check=True,
                    )
            return sp

        def emit_fin(hp, ib, jc, sp, po, njc):
            s = 128 * max(0, jc - 4 * ib)
            e = epool.tile([128, 2, 512], BF, tag="e", name=f"e{hp}_{ib}_{jc}")
            nc.scalar.activation(
                e[:, :, s:512], sp[:, :, s:512], AF.Exp,
                scale=1.0 / math.sqrt(HD), bias=ebias_b[:, :],
            )
            first, last = (jc == 0), (jc == njc - 1)
            for h in range(2):
                head = hp * 2 + h
                nc.tensor.matmul(
                    po[h][:, s:512], v4[:, jc, head, :], e[:, h, s:512],
                    start=first, stop=last,
                )

        def emit_norm(hp, ib, po):
            isl = slice(ib * 512, (ib + 1) * 512)
            rv = rvp.tile([128, 1024], BF, tag="rv", name=f"rv{hp}_{ib}")
            for h in range(2):
                with nc.allow_low_precision(reason="bf16 softmax denom"):
                    nc.vector.reciprocal(
                        rv[64:65, h * 512:(h + 1) * 512], po[h][64:65, :]
                    )
            pos = [rvp.tile([65, 512], F32, tag="pos", name=f"pos{hp}_{ib}_{h}")
                   for h in range(2)]
            nc.scalar.copy(out=pos[0], in_=po[0])
            nc.vector.tensor_copy(pos[1], po[1])
            pb = [pop.tile([64, 512], F32, tag="po", name=f"pb{hp}_{ib}_{h}")
                  for h in range(2)]
            for h in range(2):
                nc.tensor.matmul(
                    pb[h], onesP[64:65, 0:64], rv[64:65, h * 512:(h + 1) * 512],
                    start=True, stop=True,
                )
            nc.vector.tensor_mul(ont[0:64, hp, isl], pos[0][0:64, :], pb[0])
            stage = stp.tile([64, 512], BF, tag="stage", name=f"st{hp}_{ib}")
            nc.vector.tensor_mul(stage, pos[1][0:64, :], pb[1])
            eng = nc.scalar if (ib == NIB - 1 and hp == 1) else nc.sync
            eng.dma_start(out=ont[64:128, hp, isl], in_=stage)

        def emit_cfull(tb, tail=False):
            # whole 128-token output block: the two 512-col halves share their
            # stationary ont blocks (hp-major matmul order)
            ts = slice(tb * 128, (tb + 1) * 128)
            pj = [ps1.tile([128, 512], F32, tag="ps1", name=f"pj{tb}_{nh}")
                  for nh in range(2)]
            for hp in range(2):
                for nh in range(2):
                    nc.tensor.matmul(
                        pj[nh], ont[:, hp, ts], wp_sb[:, hp, nh * 512:(nh + 1) * 512],
                        start=(hp == 0), stop=(hp == 1), skip_group_check=True,
                    )
            ob = stp.tile([128, D], BF, tag="ob", name=f"ob{tb}")
            if tail:
                # split copies across engines and DMA each half as soon as it
                # lands: shortens the end-of-kernel chain
                nc.vector.tensor_copy(ob[:, 0:512], pj[0])
                nc.sync.dma_start(out=outp[ts, 0:512], in_=ob[:, 0:512])
                nc.scalar.copy(out=ob[:, 512:1024], in_=pj[1])
                nc.scalar.dma_start(out=outp[ts, 512:1024], in_=ob[:, 512:1024])
            else:
                nc.vector.tensor_copy(ob[:, 0:512], pj[0])
                nc.vector.tensor_copy(ob[:, 512:1024], pj[1])
                nc.sync.dma_start(out=outp[ts, :], in_=ob)

        # ---- phase A: QKV projection pipeline; transposes trail two blocks.
        # i-block 0's attention (short pipelines, Act otherwise idle here) is
        # woven one step per t-block into A's back half; the last two
        # transposes (only needed by i-block 3) are deferred into phase B so
        # they don't head-of-line-block the PE queue while the DVE rope chain
        # for the last t-blocks drains. ----
        qrs = {}
        ib0 = {}

        def ib0_unit(i):
            hp, u = divmod(i, 5)
            if u == 0:
                ib0['po', hp] = [
                    pop.tile([65, 512], F32, tag="po", name=f"po{hp}_0_{h}")
                    for h in range(2)]
                ib0['sp', hp, 0] = emit_sp(hp, 0, 0)
            elif u <= 3:
                sp_cur = emit_sp(hp, 0, u)
                emit_fin(hp, 0, u - 1, ib0.pop(('sp', hp, u - 1)),
                         ib0['po', hp], 4)
                ib0['sp', hp, u] = sp_cur
            else:
                emit_fin(hp, 0, 3, ib0.pop(('sp', hp, 3)), ib0['po', hp], 4)
                emit_norm(hp, 0, ib0.pop(('po', hp)))

        for tb in range(NTB):
            if tb >= 2:
                emit_tr(tb - 2, qrs.pop(tb - 2))
            qrs[tb] = emit_qkv(tb)
            if tb >= 6:
                ib0_unit(tb - 6)
        deferred_tr = [NTB - 2, NTB - 1]

        # ---- phase B: attention (jc-pipelined), with C woven in ----
        for ib in range(1, NIB):
            njc = 4 * ib + 4
            blocks = list(range(4 * (ib - 1), 4 * ib)) if ib > 0 else []
            bidx = 0
            for hp in range(2):
                po = [pop.tile([65, 512], F32, tag="po", name=f"po{hp}_{ib}_{h}")
                      for h in range(2)]
                sp_prev = emit_sp(hp, ib, 0)
                if deferred_tr:
                    emit_tr(deferred_tr[0], qrs.pop(deferred_tr.pop(0)))
                for jc in range(1, njc):
                    sp_cur = emit_sp(hp, ib, jc)
                    emit_fin(hp, ib, jc - 1, sp_prev, po, njc)
                    sp_prev = sp_cur
                    if jc % 3 == 1 and bidx < len(blocks):
                        emit_cfull(blocks[bidx])
                        bidx += 1
                emit_fin(hp, ib, njc - 1, sp_prev, po, njc)
                emit_norm(hp, ib, po)
            while bidx < len(blocks):
                emit_cfull(blocks[bidx])
                bidx += 1
        for tb in range(12, 16):
            emit_cfull(tb, tail=True)

    nc.compile()
    return nc


def host_inputs(x, w_attn, b_attn, w_proj, g_q, g_k, rope_cos, rope_sin):
    """Per-core input maps + exp bias."""
    import ml_dtypes
    bf16 = ml_dtypes.bfloat16
    x = np.asarray(x, dtype=np.float32)
    w_attn = np.asarray(w_attn, dtype=np.float32)
    b_attn = np.asarray(b_attn, dtype=np.float32)
    w_proj = np.asarray(w_proj, dtype=np.float32)
    g_q = np.asarray(g_q, dtype=np.float32)
    g_k = np.asarray(g_k, dtype=np.float32)
    rope_cos = np.asarray(rope_cos, dtype=np.float32)
    rope_sin = np.asarray(rope_sin, dtype=np.float32)

    # |s| <= 8 * max|g_q| * max|g_k| after RMSNorm; subtract for exp safety
    bound = 8.0 * max(1e-6, float(np.abs(g_q).max())) * max(
        1e-6, float(np.abs(g_k).max())
    )
    exp_bias = -bound

    # rope tables with gains folded in; shifted-sign sin for rotate_half
    def sg_of(g):
        sgn = np.where(np.arange(HD) < HD // 2, -1.0, 1.0).astype(np.float32)
        gperm = np.roll(g, HD // 2)  # g[(c+32)%64]
        return rope_sin * (sgn * gperm)[None, :]  # [T, HD]

    cgq = rope_cos * g_q[None, :]
    cgk = rope_cos * g_k[None, :]
    sgq = sg_of(g_q)
    sgk = sg_of(g_k)

    def arrange_rope(a_q, a_k):
        # [T, HD] x2 -> [128, NTB*128] with [p, tb, {q:64 | k:64}]
        aq = a_q.reshape(NTB, 128, HD).transpose(1, 0, 2)
        ak = a_k.reshape(NTB, 128, HD).transpose(1, 0, 2)
        return np.ascontiguousarray(
            np.concatenate([aq, ak], axis=2).reshape(128, NTB * 128)
        ).astype(np.float32)

    cg_arr = arrange_rope(cgq, cgk)
    sg_arr = arrange_rope(sgq, sgk)

    ident = np.eye(128, dtype=np.float32)
    ones = np.ones((128, 128), dtype=np.float32)
    # maskT[p, j] = MASKVAL for j > p: the mask matmul (stationary maskT,
    # moving identity) adds MASKVAL at sp[j, i] for key j > query i
    maskT = MASKVAL * np.triu(np.ones((128, 128), np.float32), 1)

    in_maps = []
    for c in range(NCORES):
        b, hg = divmod(c, 4)
        f0 = hg * F
        rows = np.concatenate([
            np.arange(f0, f0 + F),
            D + np.arange(f0, f0 + F),
            2 * D + np.arange(f0, f0 + F),
        ])
        w = w_attn[rows]                      # [768, 1024]
        wqkvT = np.ascontiguousarray(w.T)     # [1024, 768]
        wqkv8 = wqkvT.reshape(8, 128, 3 * F).transpose(1, 0, 2).reshape(128, 8 * 3 * F)
        wpT = np.ascontiguousarray(w_proj[:, f0:f0 + F].T)  # [256, 1024]
        wp2 = wpT.reshape(2, 128, D).transpose(1, 0, 2).reshape(128, 2 * D)
        wb = np.concatenate(
            [wqkv8, wp2, cg_arr, sg_arr, ident, ones, maskT], axis=1
        ).astype(bf16)
        assert wb.shape == (128, WBC)
        xtT = np.ascontiguousarray(x[b].T)    # [1024, 2048]
        xt8 = np.ascontiguousarray(
            xtT.reshape(8, 128, T).transpose(1, 0, 2).reshape(128, 8 * T)
        ).astype(bf16)
        m = {"xt8": xt8, "wb": np.ascontiguousarray(wb)}
        if bool(np.any(b_attn)):
            m["bqkv"] = np.ascontiguousarray(b_attn[rows].reshape(1, 3 * F))
        in_maps.append(m)
    return in_maps, exp_bias


_CACHE = {}


def kernel(x, w_attn, b_attn, w_proj, b_proj, g_q, g_k, rope_cos, rope_sin):
    in_maps, exp_bias = host_inputs(
        x, w_attn, b_attn, w_proj, g_q, g_k, rope_cos, rope_sin
    )
    has_bias = bool(np.any(np.asarray(b_attn)))
    key = (float(exp_bias), has_bias)
    if key not in _CACHE:
        _CACHE[key] = build_program(exp_bias, has_bias)
    nc = _CACHE[key]
    res = run_bass_kernel_spmd(nc, in_maps, list(range(NCORES)))
    out = np.zeros((B, T, D), dtype=np.float32)
    for c in range(NCORES):
        out[c // 4] += np.asarray(res.results[c]["outp"], dtype=np.float32)
    out += np.asarray(b_proj, dtype=np.float32)[None, None, :]
    return out
